# revision 48
# baseline (speedup 1.0000x reference)
"""AdditiveAttention (Bahdanau) distributed Bass kernel for 8 TRN2 NeuronCores.

Reference computation (per batch b):
    qp = queries[b] @ W_q                  # [Q, H]
    kp = keys[b]    @ W_k                  # [K, H]
    S[q,k]  = sum_h w_v[h] * tanh(qp[q,h] + kp[k,h])
    S masked to -1e6 for k >= valid_lens[b]
    attn = softmax(S, axis=k)
    out[b] = attn @ values[b]              # [Q, DV]

Strategy: the K axis is cut into C=128-wide chunks; only chunks with
k < ceil(valid_len/C)*C are generated (the rest are fully masked -> exp==0
-> contribute nothing). All chunks across all batches are distributed evenly
over the 8 cores (M chunks per core, padded with fully-masked dummy chunks).
Each chunk produces a partial softmax numerator (sum_k exp(S)*V) and
denominator (sum_k exp(S)); the host sums partials per batch and divides.
Because scores are bounded (|S| <= sum|w_v| ~ 9), exp() cannot overflow and
no max-subtraction is needed, so partial sums combine exactly.

Device pipeline per chunk:
  1. Projections produce "combo" stationaries [qp rows 0..63 | kp rows
     0..63] directly in the right PSUM partitions via tile_position column
     offsets (out.base_partition 0 / 64).
  2. Z[h, (q,k)] = qp[q,h] + kp[k,h] is ONE matmul per 512 columns:
     stationary = combo (bf16), moving = a constant two-hot selector
     matrix (twohot[sym, (qq,kk)] = 1 at sym=qq and sym=64+kk).
  3. ACT reads Z straight from PSUM: tanh -> F (bf16, SBUF).
  4. S^T[k,q] = sum_h w_v[h]*F[h,q,k]: per-q matmul, stationary=F slice,
     moving = w_v column (LDW/MM pairs pipeline at ~25ns).
  5. E = exp(S^T + mask_bias) (per-partition bias, no max subtraction).
  6. [numer | denom] = E^T.T @ [V | 1] in one matmul.
Host divides summed numerators by summed denominators.
"""

import math
import os

import numpy as np

import concourse.bacc as bacc
import concourse.bass as bass
import concourse.tile as tile
from concourse import mybir
from concourse.bass_utils import run_bass_kernel_spmd

B, Q, K, QS, KS, H, DV = 16, 128, 512, 256, 256, 128, 256
N_CORES = 8
C = int(os.environ.get("KERNEL_CHUNK", "128"))  # k-chunk size
MASK_NEG = -30000.0  # exp(S + MASK_NEG) == 0.0 exactly in f32 (|S| <= ~9)

F32 = mybir.dt.float32
BF16 = mybir.dt.bfloat16

_BUILD_CACHE: dict = {}
LAST_RESULT = None  # BassKernelResults of the most recent run (for timing)


def _build(M: int) -> bass.Bass:
    """Build the per-core program processing M independent k-chunks."""
    nc = bacc.Bacc()

    # Host-packed per-core inputs. qT/kT are pre-transposed on host:
    # qT[i] = queries[b_i].T  ([QS, Q]), kT[i] = keys[b_i, o:o+C].T ([KS, C]).
    qT = nc.declare_dram_parameter("qT", [M, QS, Q], F32, isOutput=False)
    kT = nc.declare_dram_parameter("kT", [M, KS, C], F32, isOutput=False)
    vp = nc.declare_dram_parameter("vp", [M, C, DV], F32, isOutput=False)
    # maskb[k, i]: 0.0 where chunk i position k is valid else MASK_NEG
    mb = nc.declare_dram_parameter("mb", [C, M], F32, isOutput=False)
    wq = nc.declare_dram_parameter("wq", [QS, H], F32, isOutput=False)
    wk = nc.declare_dram_parameter("wk", [KS, H], F32, isOutput=False)
    wv = nc.declare_dram_parameter("wv", [H, 1], F32, isOutput=False)
    twohot = nc.declare_dram_parameter("twohot", [128, 4096], BF16, isOutput=False)
    onum = nc.declare_dram_parameter("onum", [M, Q, DV], F32, isOutput=True)
    oden = nc.declare_dram_parameter("oden", [M, Q], F32, isOutput=True)

    ND = QS // 128  # 128-row blocks in the contraction dim

    with tile.TileContext(nc) as tc:
        with (
            tc.tile_pool(name="consts", bufs=1) as consts,
            tc.tile_pool(name="stg", bufs=M) as stg,
            tc.tile_pool(name="io", bufs=2) as io,
            tc.tile_pool(name="mid", bufs=2) as mid,
            tc.tile_pool(name="zbuf", bufs=2) as zbuf,
            tc.tile_pool(name="pcmb", bufs=2, space="PSUM") as pcmb,
            tc.tile_pool(name="pz", bufs=2, space="PSUM") as pz,
            tc.tile_pool(name="pout", bufs=2, space="PSUM") as pout,
        ):
            # ---- one-time constants ----
            # Every tile read by the PE is staged through a DVE copy so that
            # PE instructions only ever wait on the DVE (or ACT) semaphore —
            # PE (hw-decoded) instructions support a single sync wait.
            wq_stg = consts.tile([128, ND, H], F32)
            nc.sync.dma_start(out=wq_stg, in_=wq.rearrange("(n p) h -> p n h", p=128))
            wq_bf = consts.tile([128, ND, H], BF16)
            nc.vector.tensor_copy(wq_bf, wq_stg)
            wk_stg = consts.tile([128, ND, H], F32)
            nc.sync.dma_start(out=wk_stg, in_=wk.rearrange("(n p) h -> p n h", p=128))
            wk_bf = consts.tile([128, ND, H], BF16)
            nc.vector.tensor_copy(wk_bf, wk_stg)
            wv_sb = consts.tile([H, 1], F32)
            nc.sync.dma_start(out=wv_sb, in_=wv[:])
            wv_bf = consts.tile([H, 1], BF16)
            nc.vector.tensor_copy(wv_bf, wv_sb)
            mb_sb = consts.tile([C, M], F32)
            nc.sync.dma_start(out=mb_sb, in_=mb[:])
            th_stg = consts.tile([128, 4096], BF16)
            for j in range(4):
                nc.sync.dma_start(
                    out=th_stg[:, j * 1024 : (j + 1) * 1024],
                    in_=twohot[:, j * 1024 : (j + 1) * 1024],
                )
            th_bf = consts.tile([128, 4096], BF16)
            nc.vector.tensor_copy(th_bf, th_stg)

            for i in range(M):
                # ---- DMA chunk inputs + DVE staging ----
                v_in = stg.tile([C, DV], F32, tag="vin")
                nc.sync.dma_start(out=v_in, in_=vp[i])
                # V in bf16 with a ones column appended (for the denominator)
                v_bf = mid.tile([C, DV + 1], BF16, tag="vbf")
                nc.vector.tensor_copy(v_bf[:, :DV], v_in)
                nc.vector.memset(v_bf[:, DV : DV + 1], 1.0)

                qT_in = qT[i].rearrange("(n p) q -> p n q", p=128)
                kT_in = kT[i].rearrange("(n p) k -> p n k", p=128)
                qT_bf, kT_bf = [], []
                for n in range(ND):
                    qs = stg.tile([128, Q], F32, tag=f"qTs{n}")
                    nc.sync.dma_start(out=qs, in_=qT_in[:, n, :])
                    qb = io.tile([128, Q], BF16, tag=f"qT{n}")
                    nc.vector.tensor_copy(qb, qs)
                    qT_bf.append(qb)
                    ks = stg.tile([128, C], F32, tag=f"kTs{n}")
                    nc.sync.dma_start(out=ks, in_=kT_in[:, n, :])
                    kb = io.tile([128, C], BF16, tag=f"kT{n}")
                    nc.vector.tensor_copy(kb, ks)
                    kT_bf.append(kb)

                f_bf = zbuf.tile([H, Q, C], BF16, tag="f")

                # ---- per 64q x 64k block: combo projection + Z + tanh ----
                for qh in range(2):
                    for kh in range(C // 64):
                        combo_ps = pcmb.tile([128, H], F32, tag="cst")
                        for n in range(ND):
                            nc.tensor.matmul(
                                combo_ps[0:64, :],
                                lhsT=qT_bf[n][:, 64 * qh : 64 * qh + 64],
                                rhs=wq_bf[:, n, :],
                                start=(n == 0),
                                stop=(n == ND - 1),
                            )
                        for n in range(ND):
                            nc.tensor.matmul(
                                combo_ps[64:128, :],
                                lhsT=kT_bf[n][:, 64 * kh : 64 * kh + 64],
                                rhs=wk_bf[:, n, :],
                                start=(n == 0),
                                stop=(n == ND - 1),
                            )
                        combo_bf = mid.tile([128, H], BF16, tag="combo")
                        nc.vector.tensor_copy(combo_bf, combo_ps)

                        for tq in range(4):  # 16 q values per z tile
                            z_ps = pz.tile([H, 1024], F32, tag="z")
                            for hb in range(2):
                                nc.tensor.matmul(
                                    z_ps[:, hb * 512 : (hb + 1) * 512],
                                    lhsT=combo_bf,
                                    rhs=th_bf[
                                        :, tq * 1024 + hb * 512 : tq * 1024 + (hb + 1) * 512
                                    ],
                                    start=True,
                                    stop=True,
                                )
                            nc.scalar.activation(
                                out=f_bf[
                                    :,
                                    64 * qh + 16 * tq : 64 * qh + 16 * tq + 16,
                                    64 * kh : 64 * kh + 64,
                                ],
                                in_=z_ps,
                                func=mybir.ActivationFunctionType.Tanh,
                            )

                # ---- S^T[k, q] = sum_h w_v[h] * F[h, q, k]  (PE, per-q) ----
                st_ps = pcmb.tile([C, Q], F32, tag="cst")
                for q in range(Q):
                    nc.tensor.matmul(
                        st_ps[:, q : q + 1],
                        lhsT=f_bf[:, q, :],
                        rhs=wv_bf,
                        start=True,
                        stop=True,
                    )

                # ---- E^T = exp(S^T + mask) ----
                e_bf = mid.tile([C, Q], BF16, tag="e")
                nc.scalar.activation(
                    out=e_bf,
                    in_=st_ps,
                    func=mybir.ActivationFunctionType.Exp,
                    bias=mb_sb[:, i : i + 1],
                )

                # ---- [numer | denom] = E^T.T @ [V | 1]  (one matmul) ----
                o_ps = pout.tile([Q, DV + 1], F32, tag="o")
                nc.tensor.matmul(o_ps, lhsT=e_bf, rhs=v_bf, start=True, stop=True)

                o_sb = io.tile([Q, DV + 1], F32, tag="osb")
                nc.vector.tensor_copy(o_sb, o_ps)
                half = DV // 2
                nc.sync.dma_start(out=onum[i][:, :half], in_=o_sb[:, :half])
                nc.sync.dma_start(out=onum[i][:, half:DV], in_=o_sb[:, half:DV])
                nc.sync.dma_start(out=oden[i], in_=o_sb[:, DV : DV + 1])

    nc.finalize()
    return nc


def _twohot() -> np.ndarray:
    import ml_dtypes

    t = np.zeros((128, 4096), ml_dtypes.bfloat16)
    cols = np.arange(4096)
    qq, kk = cols // 64, cols % 64
    t[qq, cols] = 1.0
    t[64 + kk, cols] = 1.0
    return t


def kernel(queries, keys, values, valid_lens, W_q, W_k, w_v):
    queries = np.ascontiguousarray(np.asarray(queries, dtype=np.float32))
    keys = np.ascontiguousarray(np.asarray(keys, dtype=np.float32))
    values = np.ascontiguousarray(np.asarray(values, dtype=np.float32))
    W_q = np.ascontiguousarray(np.asarray(W_q, dtype=np.float32))
    W_k = np.ascontiguousarray(np.asarray(W_k, dtype=np.float32))
    w_v = np.ascontiguousarray(np.asarray(w_v, dtype=np.float32))
    vl = np.asarray(valid_lens).astype(np.int64)

    # ---- chunk list: (batch, k_offset) for every live k-chunk ----
    chunks = []
    for b in range(B):
        nch = max(1, int(math.ceil(float(vl[b]) / C)))
        for j in range(nch):
            chunks.append((b, j * C))
    M = int(math.ceil(len(chunks) / N_CORES))
    n_pad = N_CORES * M - len(chunks)
    chunks += [None] * n_pad  # dummy, fully masked chunks

    nc = _BUILD_CACHE.get(M)
    if nc is None:
        nc = _build(M)
        _BUILD_CACHE[M] = nc

    # ---- host-side packing ----
    kidx = np.arange(C)
    twohot = _twohot()
    in_maps = []
    core_chunks = []
    for c in range(N_CORES):
        my = chunks[c * M : (c + 1) * M]
        core_chunks.append(my)
        qTp = np.zeros((M, QS, Q), np.float32)
        kTp = np.zeros((M, KS, C), np.float32)
        vpp = np.zeros((M, C, DV), np.float32)
        mbp = np.full((C, M), MASK_NEG, np.float32)
        for i, ch in enumerate(my):
            if ch is None:
                continue
            b, o = ch
            qTp[i] = queries[b].T
            kTp[i] = keys[b, o : o + C].T
            vpp[i] = values[b, o : o + C]
            mbp[:, i] = np.where(o + kidx < vl[b], 0.0, MASK_NEG)
        in_maps.append(
            {
                "qT": qTp,
                "kT": kTp,
                "vp": vpp,
                "mb": np.ascontiguousarray(mbp),
                "wq": W_q,
                "wk": W_k,
                "wv": np.ascontiguousarray(w_v.reshape(H, 1)),
                "twohot": twohot,
            }
        )

    global LAST_RESULT
    res = run_bass_kernel_spmd(
        nc,
        in_maps,
        core_ids=list(range(N_CORES)),
        trace=bool(os.environ.get("KERNEL_TRACE")),
    )
    LAST_RESULT = res

    # ---- host combine: sum partials per batch, divide ----
    num = np.zeros((B, Q, DV), np.float64)
    den = np.zeros((B, Q), np.float64)
    for c in range(N_CORES):
        onum = res.results[c]["onum"]
        oden = res.results[c]["oden"]
        for i, ch in enumerate(core_chunks[c]):
            if ch is None:
                continue
            b, _ = ch
            num[b] += onum[i]
            den[b] += oden[i]
    return (num / den[:, :, None]).astype(np.float32)


# revision 49
# speedup vs baseline: 1.1231x; 1.1231x over previous
"""AdditiveAttention (Bahdanau) distributed Bass kernel for 8 TRN2 NeuronCores.

Reference computation (per batch b):
    qp = queries[b] @ W_q                  # [Q, H]
    kp = keys[b]    @ W_k                  # [K, H]
    S[q,k]  = sum_h w_v[h] * tanh(qp[q,h] + kp[k,h])
    S masked to -1e6 for k >= valid_lens[b]
    attn = softmax(S, axis=k)
    out[b] = attn @ values[b]              # [Q, DV]

Strategy: the K axis is cut into C=128-wide chunks; only chunks with
k < ceil(valid_len/C)*C are generated (the rest are fully masked -> exp==0
-> contribute nothing). All chunks across all batches are distributed evenly
over the 8 cores (M chunks per core, padded with fully-masked dummy chunks).
Each chunk produces a partial softmax numerator (sum_k exp(S)*V) and
denominator (sum_k exp(S)); the host sums partials per batch and divides.
Because scores are bounded (|S| <= sum|w_v| ~ 9), exp() cannot overflow and
no max-subtraction is needed, so partial sums combine exactly.

Device pipeline per chunk:
  1. Projections produce "combo" stationaries [qp rows 0..63 | kp rows
     0..63] directly in the right PSUM partitions via tile_position column
     offsets (out.base_partition 0 / 64).
  2. Z[h, (q,k)] = qp[q,h] + kp[k,h] is ONE matmul per 512 columns:
     stationary = combo (bf16), moving = a constant two-hot selector
     matrix (twohot[sym, (qq,kk)] = 1 at sym=qq and sym=64+kk).
  3. ACT reads Z straight from PSUM: tanh -> F (bf16, SBUF).
  4. S^T[k,q] = sum_h w_v[h]*F[h,q,k]: per-q matmul, stationary=F slice,
     moving = w_v column (LDW/MM pairs pipeline at ~25ns).
  5. E = exp(S^T + mask_bias) (per-partition bias, no max subtraction).
  6. [numer | denom] = E^T.T @ [V | 1] in one matmul.
Host divides summed numerators by summed denominators.
"""

import math
import os

import numpy as np

import concourse.bacc as bacc
import concourse.bass as bass
import concourse.tile as tile
from concourse import mybir
from concourse.bass_utils import run_bass_kernel_spmd

B, Q, K, QS, KS, H, DV = 16, 128, 512, 256, 256, 128, 256
N_CORES = 8
C = int(os.environ.get("KERNEL_CHUNK", "128"))  # k-chunk size
MASK_NEG = -30000.0  # exp(S + MASK_NEG) == 0.0 exactly in f32 (|S| <= ~9)

F32 = mybir.dt.float32
BF16 = mybir.dt.bfloat16

_BUILD_CACHE: dict = {}
LAST_RESULT = None  # BassKernelResults of the most recent run (for timing)


def _build(M: int) -> bass.Bass:
    """Build the per-core program processing M independent k-chunks."""
    nc = bacc.Bacc()

    # Host-packed per-core inputs. qT/kT are pre-transposed on host:
    # qT[i] = queries[b_i].T  ([QS, Q]), kT[i] = keys[b_i, o:o+C].T ([KS, C]).
    qT = nc.declare_dram_parameter("qT", [M, QS, Q], F32, isOutput=False)
    kT = nc.declare_dram_parameter("kT", [M, KS, C], F32, isOutput=False)
    vp = nc.declare_dram_parameter("vp", [M, C, DV], F32, isOutput=False)
    # maskb[k, i]: 0.0 where chunk i position k is valid else MASK_NEG
    mb = nc.declare_dram_parameter("mb", [C, M], F32, isOutput=False)
    wq = nc.declare_dram_parameter("wq", [QS, H], F32, isOutput=False)
    wk = nc.declare_dram_parameter("wk", [KS, H], F32, isOutput=False)
    wv = nc.declare_dram_parameter("wv", [H, 1], F32, isOutput=False)
    twohot = nc.declare_dram_parameter("twohot", [128, 4096], BF16, isOutput=False)
    onum = nc.declare_dram_parameter("onum", [M, Q, DV], F32, isOutput=True)
    oden = nc.declare_dram_parameter("oden", [M, Q], F32, isOutput=True)

    ND = QS // 128  # 128-row blocks in the contraction dim

    with tile.TileContext(nc) as tc:
        with (
            tc.tile_pool(name="consts", bufs=1) as consts,
            tc.tile_pool(name="stg", bufs=M) as stg,
            tc.tile_pool(name="io", bufs=2) as io,
            tc.tile_pool(name="mid", bufs=2) as mid,
            tc.tile_pool(name="zbuf", bufs=2) as zbuf,
            tc.tile_pool(name="pcmb", bufs=2, space="PSUM") as pcmb,
            tc.tile_pool(name="pz", bufs=2, space="PSUM") as pz,
            tc.tile_pool(name="pout", bufs=2, space="PSUM") as pout,
        ):
            # ---- one-time constants ----
            # Every tile read by the PE is staged through a DVE copy so that
            # PE instructions only ever wait on the DVE (or ACT) semaphore —
            # PE (hw-decoded) instructions support a single sync wait.
            wq_stg = consts.tile([128, ND, H], F32)
            nc.sync.dma_start(out=wq_stg, in_=wq.rearrange("(n p) h -> p n h", p=128))
            wq_bf = consts.tile([128, ND, H], BF16)
            nc.vector.tensor_copy(wq_bf, wq_stg)
            wk_stg = consts.tile([128, ND, H], F32)
            nc.sync.dma_start(out=wk_stg, in_=wk.rearrange("(n p) h -> p n h", p=128))
            wk_bf = consts.tile([128, ND, H], BF16)
            nc.vector.tensor_copy(wk_bf, wk_stg)
            wv_sb = consts.tile([H, 1], F32)
            nc.sync.dma_start(out=wv_sb, in_=wv[:])
            wv_bf = consts.tile([H, 1], BF16)
            nc.vector.tensor_copy(wv_bf, wv_sb)
            mb_sb = consts.tile([C, M], F32)
            nc.sync.dma_start(out=mb_sb, in_=mb[:])
            th_stg = consts.tile([128, 4096], BF16)
            for j in range(4):
                nc.sync.dma_start(
                    out=th_stg[:, j * 1024 : (j + 1) * 1024],
                    in_=twohot[:, j * 1024 : (j + 1) * 1024],
                )
            th_bf = consts.tile([128, 4096], BF16)
            nc.vector.tensor_copy(th_bf, th_stg)

            for i in range(M):
                # ---- DMA chunk inputs + DVE staging ----
                v_in = stg.tile([C, DV], F32, tag="vin")
                nc.sync.dma_start(out=v_in, in_=vp[i])
                # V in bf16 with a ones column appended (for the denominator)
                v_bf = mid.tile([C, DV + 1], BF16, tag="vbf")
                nc.vector.tensor_copy(v_bf[:, :DV], v_in)
                nc.vector.memset(v_bf[:, DV : DV + 1], 1.0)

                qT_in = qT[i].rearrange("(n p) q -> p n q", p=128)
                kT_in = kT[i].rearrange("(n p) k -> p n k", p=128)
                qT_bf, kT_bf = [], []
                for n in range(ND):
                    qs = stg.tile([128, Q], F32, tag=f"qTs{n}")
                    nc.sync.dma_start(out=qs, in_=qT_in[:, n, :])
                    qb = io.tile([128, Q], BF16, tag=f"qT{n}")
                    nc.vector.tensor_copy(qb, qs)
                    qT_bf.append(qb)
                    ks = stg.tile([128, C], F32, tag=f"kTs{n}")
                    nc.sync.dma_start(out=ks, in_=kT_in[:, n, :])
                    kb = io.tile([128, C], BF16, tag=f"kT{n}")
                    nc.vector.tensor_copy(kb, ks)
                    kT_bf.append(kb)

                f_bf = zbuf.tile([H, Q, C], BF16, tag="f", bufs=3)

                # ---- per 64q x 64k block: combo projection + Z + tanh ----
                for qh in range(2):
                    for kh in range(C // 64):
                        combo_ps = pcmb.tile([128, H], F32, tag="cst")
                        for n in range(ND):
                            nc.tensor.matmul(
                                combo_ps[0:64, :],
                                lhsT=qT_bf[n][:, 64 * qh : 64 * qh + 64],
                                rhs=wq_bf[:, n, :],
                                start=(n == 0),
                                stop=(n == ND - 1),
                            )
                        for n in range(ND):
                            nc.tensor.matmul(
                                combo_ps[64:128, :],
                                lhsT=kT_bf[n][:, 64 * kh : 64 * kh + 64],
                                rhs=wk_bf[:, n, :],
                                start=(n == 0),
                                stop=(n == ND - 1),
                            )
                        combo_bf = mid.tile([128, H], BF16, tag="combo")
                        nc.vector.tensor_copy(combo_bf, combo_ps)

                        for tq in range(4):  # 16 q values per z tile
                            z_ps = pz.tile([H, 1024], F32, tag="z")
                            for hb in range(2):
                                nc.tensor.matmul(
                                    z_ps[:, hb * 512 : (hb + 1) * 512],
                                    lhsT=combo_bf,
                                    rhs=th_bf[
                                        :, tq * 1024 + hb * 512 : tq * 1024 + (hb + 1) * 512
                                    ],
                                    start=True,
                                    stop=True,
                                )
                            nc.scalar.activation(
                                out=f_bf[
                                    :,
                                    64 * qh + 16 * tq : 64 * qh + 16 * tq + 16,
                                    64 * kh : 64 * kh + 64,
                                ],
                                in_=z_ps,
                                func=mybir.ActivationFunctionType.Tanh,
                            )

                # ---- S^T[k, q] = sum_h w_v[h] * F[h, q, k]  (PE, per-q) ----
                st_ps = pcmb.tile([C, Q], F32, tag="cst")
                for q in range(Q):
                    nc.tensor.matmul(
                        st_ps[:, q : q + 1],
                        lhsT=f_bf[:, q, :],
                        rhs=wv_bf,
                        start=True,
                        stop=True,
                    )

                # ---- E^T = exp(S^T + mask) ----
                e_bf = mid.tile([C, Q], BF16, tag="e")
                nc.scalar.activation(
                    out=e_bf,
                    in_=st_ps,
                    func=mybir.ActivationFunctionType.Exp,
                    bias=mb_sb[:, i : i + 1],
                )

                # ---- [numer | denom] = E^T.T @ [V | 1]  (one matmul) ----
                o_ps = pout.tile([Q, DV + 1], F32, tag="o")
                nc.tensor.matmul(o_ps, lhsT=e_bf, rhs=v_bf, start=True, stop=True)

                o_sb = io.tile([Q, DV + 1], F32, tag="osb")
                nc.vector.tensor_copy(o_sb, o_ps)
                half = DV // 2
                nc.sync.dma_start(out=onum[i][:, :half], in_=o_sb[:, :half])
                nc.sync.dma_start(out=onum[i][:, half:DV], in_=o_sb[:, half:DV])
                nc.sync.dma_start(out=oden[i], in_=o_sb[:, DV : DV + 1])

    nc.finalize()
    return nc


def _twohot() -> np.ndarray:
    import ml_dtypes

    t = np.zeros((128, 4096), ml_dtypes.bfloat16)
    cols = np.arange(4096)
    qq, kk = cols // 64, cols % 64
    t[qq, cols] = 1.0
    t[64 + kk, cols] = 1.0
    return t


def kernel(queries, keys, values, valid_lens, W_q, W_k, w_v):
    queries = np.ascontiguousarray(np.asarray(queries, dtype=np.float32))
    keys = np.ascontiguousarray(np.asarray(keys, dtype=np.float32))
    values = np.ascontiguousarray(np.asarray(values, dtype=np.float32))
    W_q = np.ascontiguousarray(np.asarray(W_q, dtype=np.float32))
    W_k = np.ascontiguousarray(np.asarray(W_k, dtype=np.float32))
    w_v = np.ascontiguousarray(np.asarray(w_v, dtype=np.float32))
    vl = np.asarray(valid_lens).astype(np.int64)

    # ---- chunk list: (batch, k_offset) for every live k-chunk ----
    chunks = []
    for b in range(B):
        nch = max(1, int(math.ceil(float(vl[b]) / C)))
        for j in range(nch):
            chunks.append((b, j * C))
    M = int(math.ceil(len(chunks) / N_CORES))
    n_pad = N_CORES * M - len(chunks)
    chunks += [None] * n_pad  # dummy, fully masked chunks

    nc = _BUILD_CACHE.get(M)
    if nc is None:
        nc = _build(M)
        _BUILD_CACHE[M] = nc

    # ---- host-side packing ----
    kidx = np.arange(C)
    twohot = _twohot()
    in_maps = []
    core_chunks = []
    for c in range(N_CORES):
        my = chunks[c * M : (c + 1) * M]
        core_chunks.append(my)
        qTp = np.zeros((M, QS, Q), np.float32)
        kTp = np.zeros((M, KS, C), np.float32)
        vpp = np.zeros((M, C, DV), np.float32)
        mbp = np.full((C, M), MASK_NEG, np.float32)
        for i, ch in enumerate(my):
            if ch is None:
                continue
            b, o = ch
            qTp[i] = queries[b].T
            kTp[i] = keys[b, o : o + C].T
            vpp[i] = values[b, o : o + C]
            mbp[:, i] = np.where(o + kidx < vl[b], 0.0, MASK_NEG)
        in_maps.append(
            {
                "qT": qTp,
                "kT": kTp,
                "vp": vpp,
                "mb": np.ascontiguousarray(mbp),
                "wq": W_q,
                "wk": W_k,
                "wv": np.ascontiguousarray(w_v.reshape(H, 1)),
                "twohot": twohot,
            }
        )

    global LAST_RESULT
    res = run_bass_kernel_spmd(
        nc,
        in_maps,
        core_ids=list(range(N_CORES)),
        trace=bool(os.environ.get("KERNEL_TRACE")),
    )
    LAST_RESULT = res

    # ---- host combine: sum partials per batch, divide ----
    num = np.zeros((B, Q, DV), np.float64)
    den = np.zeros((B, Q), np.float64)
    for c in range(N_CORES):
        onum = res.results[c]["onum"]
        oden = res.results[c]["oden"]
        for i, ch in enumerate(core_chunks[c]):
            if ch is None:
                continue
            b, _ = ch
            num[b] += onum[i]
            den[b] += oden[i]
    return (num / den[:, :, None]).astype(np.float32)
